# revision 1
# baseline (speedup 1.0000x reference)
"""Causal self-attention (B=2, T=2048, C=1024, H=16, D=64) on 8 trn2 NeuronCores.

Sharding: batch x head-group. Core c handles batch b = c//4 and head group
g = c%4 (4 heads = 256 channels). Per core:
  - qkv projection for its 4 heads (Q^T/K^T in [d, t] layout, V in [t, d])
  - causal flash attention for its 4 heads (scores computed K-major as S^T,
    row-sums via a ones-column appended to V, no max subtraction -- logits
    are O(1) for this problem family)
  - AllGather of Y^T across the 4 cores of the same batch (2 gathers, one
    per head pair; the first overlaps attention of the second pair, the
    second overlaps the first half of the output projection)
  - output projection for a 256-column slice of the output

Matmuls run in float32r (fp32 data truncated to FP22 internally, full PE
rate at moving dim >= 256), accumulation fp32 in PSUM.

Host gather: concatenate the 4 column slices per batch. No host math.
"""

import os
import numpy as np

import concourse.bass as bass
import concourse.bacc as bacc
import concourse.mybir as mybir
import concourse.tile as tile
from concourse import bass_utils
from concourse.bass import ds, ts
from concourse.bass_interp import get_hw_module

P = 128
B, T, C = 2, 2048, 1024
NH, D = 16, 64
NC = 8          # cores
NG = 4          # head groups (cores per batch)
HL = NH // NG   # heads per core = 4
DL = HL * D     # local channels = 256
NQ = 512        # query tile (fp32 moving-dim cap)
F32 = mybir.dt.float32
F32R = mybir.dt.float32r


def _build_body(ctx, tc, io):
    nc = tc.nc
    xt, wq, wk, wv, bq, bk, bv, wp, bp, tri, out, ytl, ytf = io
    mm = nc.tensor.matmul

    pers = ctx.enter_context(tc.tile_pool(name="pers", bufs=1))

    tri_sb = pers.tile([P, P], F32)
    nc.sync.dma_start(tri_sb[:], tri)

    qt_sb = pers.tile([P, 2, T], F32R)   # chunk j = head pair j; head 2j+1 on partitions 64..127
    kt_sb = pers.tile([P, 2, T], F32R)
    v_sb = pers.tile([P, T // P, HL, D + 1], F32R)  # [l_part, l_chunk, head, d | ones]
    ones_stage = pers.tile([P, (T // P) * HL], F32)
    nc.vector.memset(ones_stage[:], 1.0)
    nc.vector.tensor_copy(
        v_sb[:, :, :, D : D + 1],
        ones_stage[:].rearrange("p (a b) -> p a b", a=T // P)[:, :, :, None],
    )
    yth = [
        pers.tile([D, T], F32R, tag=f"yth{h}", name=f"yth{h}") for h in range(HL)
    ]

    # ---------------- qkv ----------------
    with (
        tc.tile_pool(name="xw", bufs=1) as xw,
        tc.tile_pool(name="qkvps", bufs=3, space="PSUM") as qkvps,
    ):
        xt_sb = xw.tile([P, C // P, T], F32R)
        wq_sb = xw.tile([P, C // P, DL], F32R)
        wk_sb = xw.tile([P, C // P, DL], F32R)
        wv_sb = xw.tile([P, C // P, DL], F32R)
        # chunked loads so the first matmuls start after ~1/8 of the DMA
        for cc in range(C // P):
            src = slice(P * cc, P * (cc + 1))
            nc.sync.dma_start(wq_sb[:, cc, :], wq[src, :].rearrange("p n -> p n"))
            nc.sync.dma_start(wk_sb[:, cc, :], wk[src, :].rearrange("p n -> p n"))
            nc.sync.dma_start(wv_sb[:, cc, :], wv[src, :].rearrange("p n -> p n"))
            nc.sync.dma_start(xt_sb[:, cc, :], xt[src, :].rearrange("p t -> p t"))
        bqp = xw.tile([P, 2], F32)
        nc.sync.dma_start(bqp[:], bq.rearrange("(j p) -> p j", p=P))
        bkp = xw.tile([P, 2], F32)
        nc.sync.dma_start(bkp[:], bk.rearrange("(j p) -> p j", p=P))
        bv_row = xw.tile([1, DL], F32)
        nc.sync.dma_start(bv_row[:], bv[None, :])
        bv_bc = xw.tile([P, DL], F32)
        nc.gpsimd.partition_broadcast(bv_bc[:], bv_row[:])

        # Q^T / K^T: [j, t] = sum_c W[c, j] xT[c, t]
        for w_sb, b_sb, dst in ((wq_sb, bqp, qt_sb), (wk_sb, bkp, kt_sb)):
            for j in range(2):
                for tt in range(T // NQ):
                    ps = qkvps.tile([P, NQ], F32, tag="qk")
                    for cc in range(C // P):
                        mm(
                            ps[:],
                            w_sb[:, cc, ts(j, P)],
                            xt_sb[:, cc, ts(tt, NQ)],
                            start=(cc == 0),
                            stop=(cc == C // P - 1),
                        )
                    nc.vector.tensor_scalar_add(
                        dst[:, j, ts(tt, NQ)], ps[:], b_sb[:, j : j + 1]
                    )
        # V: [t, d'] = sum_c x[t, c] Wv[c, d']
        for tt in range(T // P):
            ps = qkvps.tile([P, DL], F32, tag="v")
            for cc in range(C // P):
                mm(
                    ps[:],
                    xt_sb[:, cc, ts(tt, P)],
                    wv_sb[:, cc, :],
                    start=(cc == 0),
                    stop=(cc == C // P - 1),
                )
            nc.vector.tensor_add(
                v_sb[:, tt, :, 0:D],
                ps[:].rearrange("p (h d) -> p h d", h=HL),
                bv_bc[:].rearrange("p (h d) -> p h d", h=HL),
            )

    # ---------------- attention ----------------
    with (
        tc.tile_pool(name="pp", bufs=3) as pp,
        tc.tile_pool(name="nrm", bufs=4) as nrm,
        tc.tile_pool(name="sps", bufs=2, space="PSUM") as sps,
        tc.tile_pool(name="ops", bufs=2, space="PSUM") as ops,
    ):
        for pair in range(2):
            for qt in range(T // NQ):
                q0 = NQ * qt
                nl = q0 // P + NQ // P  # l-chunks for causal coverage
                o_ps = [
                    ops.tile([D + 1, NQ], F32, tag=f"o{hi}", name=f"o_ps{hi}")
                    for hi in range(2)
                ]

                def s_stage(lc):
                    # both heads' scores, two concurrent row-group-tiled
                    # K=64 matmuls into separate single-bank psum tiles
                    s2 = [
                        sps.tile([P, NQ], F32, tag=f"s{hi}", name=f"s{hi}")
                        for hi in range(2)
                    ]
                    for hi in range(2):
                        mm(
                            s2[hi][:],
                            kt_sb[64 * hi : 64 * hi + 64, pair, ts(lc, P)],
                            qt_sb[64 * hi : 64 * hi + 64, pair, ds(q0, NQ)],
                            start=True,
                            stop=True,
                            tile_position=(64 * hi, 0),
                        )
                    return s2

                def pv_stage(lc, s2):
                    off = P * lc - q0
                    w0 = max(off, 0)
                    pt = pp.tile([P, 2, NQ], F32R, tag="p", name="pt")
                    for hi in range(2):
                        nc.scalar.activation(
                            pt[:, hi, w0:NQ],
                            s2[hi][:, w0:NQ],
                            mybir.ActivationFunctionType.Exp,
                            bias=0.0,
                            scale=1.0 / np.sqrt(D),
                        )
                    if off >= 0:
                        for hi in range(2):
                            nc.vector.tensor_mul(
                                pt[:, hi, off : off + P],
                                pt[:, hi, off : off + P],
                                tri_sb[:],
                            )
                    for hi in range(2):
                        mm(
                            o_ps[hi][:, w0:NQ],
                            v_sb[:, lc, 2 * pair + hi, :],
                            pt[:, hi, w0:NQ],
                            start=(lc == 0),
                            stop=(lc == nl - 1),
                        )

                # software pipeline: keep one S stage ahead of exp/PV
                prev = s_stage(0)
                for lc in range(1, nl):
                    cur = s_stage(lc)
                    pv_stage(lc - 1, prev)
                    prev = cur
                pv_stage(nl - 1, prev)

                for hi in range(2):
                    h = 2 * pair + hi
                    sums_sb = nrm.tile([1, NQ], F32, tag="sums")
                    nc.vector.tensor_copy(sums_sb[:], o_ps[hi][D : D + 1, :])
                    rcp = nrm.tile([1, NQ], F32, tag="rcp")
                    nc.vector.reciprocal_approx_fast(rcp[:], sums_sb[:])
                    bc = nrm.tile([D, NQ], F32, tag="bc")
                    nc.gpsimd.partition_broadcast(bc[:], rcp[:])
                    nc.vector.tensor_mul(
                        yth[h][:, ds(q0, NQ)], o_ps[hi][0:D, :], bc[:]
                    )
            # ship this pair's Y^T and gather across the 4 cores of this batch
            nc.sync.dma_start(ytl[pair][0:D, :], yth[2 * pair][:])
            nc.sync.dma_start(ytl[pair][D : 2 * D, :], yth[2 * pair + 1][:])
            nc.gpsimd.collective_compute(
                "AllGather",
                mybir.AluOpType.bypass,
                replica_groups=[[0, 1, 2, 3], [4, 5, 6, 7]],
                ins=[ytl[pair][:]],
                outs=[ytf[pair][:]],
            )

    # ---------------- proj ----------------
    # split accumulation: pair-0 contribution right after AllGather 0 (runs
    # while AllGather 1 is still in flight), pair-1 contribution + bias after.
    with (
        tc.tile_pool(name="yf", bufs=1) as yf,
        tc.tile_pool(name="po", bufs=4) as po,
        tc.tile_pool(name="prps", bufs=4, space="PSUM") as prps,
    ):
        wp_sb = yf.tile([P, C // P, DL], F32R)
        nc.sync.dma_start(wp_sb[:], wp.rearrange("(o p) n -> p o n", p=P))
        bp_row = yf.tile([1, DL], F32)
        nc.sync.dma_start(bp_row[:], bp[None, :])
        bp_bc = yf.tile([P, DL], F32)
        nc.gpsimd.partition_broadcast(bp_bc[:], bp_row[:])
        ytf_sb = []
        for pair in range(2):
            t_ = yf.tile([P, NG, T], F32R, tag=f"ytf{pair}", name=f"ytf{pair}_sb")
            nc.sync.dma_start(t_[:], ytf[pair].rearrange("(r q) t -> q r t", q=P))
            ytf_sb.append(t_)
        acc = [
            yf.tile([P, DL], F32, tag=f"acc{tt}", name=f"acc{tt}")
            for tt in range(T // P)
        ]
        for tt in range(T // P):
            ps = prps.tile([P, DL], F32, tag="pr0")
            for r in range(NG):
                mm(
                    ps[:],
                    ytf_sb[0][:, r, ts(tt, P)],
                    wp_sb[:, 2 * r, :],
                    start=(r == 0),
                    stop=(r == NG - 1),
                )
            nc.vector.tensor_add(acc[tt][:], ps[:], bp_bc[:])
        for tt in range(T // P):
            ps = prps.tile([P, DL], F32, tag="pr1")
            for r in range(NG):
                mm(
                    ps[:],
                    ytf_sb[1][:, r, ts(tt, P)],
                    wp_sb[:, 2 * r + 1, :],
                    start=(r == 0),
                    stop=(r == NG - 1),
                )
            ot = po.tile([P, DL], F32, tag="ot")
            nc.vector.tensor_add(ot[:], ps[:], acc[tt][:])
            nc.sync.dma_start(out[ts(tt, P), :], ot[:])


def build_program():
    nc = bacc.Bacc(
        "TRN2",
        target_bir_lowering=False,
        debug=False,
        enable_asserts=False,
        num_devices=NC,
    )
    xt = nc.dram_tensor("xt", [C, T], F32R, kind="ExternalInput").ap()
    wq = nc.dram_tensor("wq", [C, DL], F32R, kind="ExternalInput").ap()
    wk = nc.dram_tensor("wk", [C, DL], F32R, kind="ExternalInput").ap()
    wv = nc.dram_tensor("wv", [C, DL], F32R, kind="ExternalInput").ap()
    bq = nc.dram_tensor("bq", [DL], F32, kind="ExternalInput").ap()
    bk = nc.dram_tensor("bk", [DL], F32, kind="ExternalInput").ap()
    bv = nc.dram_tensor("bv", [DL], F32, kind="ExternalInput").ap()
    wp = nc.dram_tensor("wp", [C, DL], F32R, kind="ExternalInput").ap()
    bp = nc.dram_tensor("bp", [DL], F32, kind="ExternalInput").ap()
    tri = nc.dram_tensor("tri", [P, P], F32, kind="ExternalInput").ap()
    out = nc.dram_tensor("out", [T, DL], F32, kind="ExternalOutput").ap()
    ytl = [
        nc.dram_tensor(f"ytl{p}", [DL // 2, T], F32R, kind="Internal").ap()
        for p in range(2)
    ]
    ytf = [
        nc.dram_tensor(f"ytf{p}", [NG * DL // 2, T], F32R, kind="Internal").ap()
        for p in range(2)
    ]
    io = (xt, wq, wk, wv, bq, bk, bv, wp, bp, tri, out, ytl, ytf)
    with tile.TileContext(nc) as tc:
        import contextlib

        with contextlib.ExitStack() as ctx:
            _build_body(ctx, tc, io)
    nc.compile()
    return nc


def make_in_maps(x, W_attn, b_attn, W_proj, b_proj):
    # scores are computed transposed (S^T[l, q]); position (l', q'') in a
    # diagonal 128x128 block is causally valid iff q'' >= l' -> upper-tri mask
    tri_np = np.triu(np.ones((P, P), dtype=np.float32))
    x = np.asarray(x, dtype=np.float32)
    W_attn = np.asarray(W_attn, dtype=np.float32)
    b_attn = np.asarray(b_attn, dtype=np.float32)
    W_proj = np.asarray(W_proj, dtype=np.float32)
    b_proj = np.asarray(b_proj, dtype=np.float32)
    in_maps = []
    for c in range(NC):
        b, g = divmod(c, NG)
        cols = slice(DL * g, DL * (g + 1))
        in_maps.append(
            {
                "xt": np.ascontiguousarray(x[b].T),
                "wq": np.ascontiguousarray(W_attn[:, cols]),
                "wk": np.ascontiguousarray(W_attn[:, C:][:, cols]),
                "wv": np.ascontiguousarray(W_attn[:, 2 * C :][:, cols]),
                "bq": np.ascontiguousarray(b_attn[cols]),
                "bk": np.ascontiguousarray(b_attn[C:][cols]),
                "bv": np.ascontiguousarray(b_attn[2 * C :][cols]),
                "wp": np.ascontiguousarray(W_proj[:, cols]),
                "bp": np.ascontiguousarray(b_proj[cols]),
                "tri": tri_np,
            }
        )
    return in_maps


_NC_CACHE = {}


def _install_ntff_hook():
    """Recreate the missing antenv.axon_hooks module so
    run_bass_kernel_spmd(trace=True) can capture NTFF profiles under axon."""
    import sys
    import types

    if "antenv.axon_hooks" in sys.modules:
        return True
    try:
        from trn_agent_boot.trn_boot import _ntff_profile_via_ctypes

        hook = _ntff_profile_via_ctypes("/opt/axon/libaxon_pjrt.so")
        if hook is None:
            return False
        mod = types.ModuleType("antenv.axon_hooks")
        mod.get_axon_ntff_profile_hook = lambda: hook
        mod.set_axon_ntff_profile_hook = lambda h: None
        sys.modules["antenv.axon_hooks"] = mod
        import antenv

        antenv.axon_hooks = mod
        # the trace path uploads artifacts to a fish bucket that doesn't
        # exist in this container; keep them local instead
        bass_utils.upload_artifacts = lambda tmpdir: tmpdir
        return True
    except Exception:
        return False


def _get_program():
    if "nc" not in _NC_CACHE:
        nc = build_program()
        nc.m = get_hw_module(nc.m)
        _NC_CACHE["nc"] = nc
    return _NC_CACHE["nc"]


def kernel(x, W_attn, b_attn, W_proj, b_proj):
    nc = _get_program()
    in_maps = make_in_maps(x, W_attn, b_attn, W_proj, b_proj)
    trace = bool(int(os.environ.get("KERNEL_TRACE", "0")))
    if trace:
        trace = _install_ntff_hook()
    res = bass_utils.run_bass_kernel_spmd(
        nc,
        in_maps,
        core_ids=list(range(NC)),
        trace=trace,
        trace_cores=list(range(NC)) if trace else None,
    )
    if trace:
        _NC_CACHE["last_results"] = res
        if res.exec_time_ns is not None:
            print(f"HW exec time: {res.exec_time_ns} ns")
            if res.instructions_and_trace is not None:
                print(f"trace: {res.instructions_and_trace[1]}")
    out = np.empty((B, T, C), dtype=np.float32)
    for c in range(NC):
        b, g = divmod(c, NG)
        out[b, :, DL * g : DL * (g + 1)] = res.results[c]["out"]
    return out



# revision 3
# speedup vs baseline: 1.5146x; 1.5146x over previous
"""Causal self-attention (B=2, T=2048, C=1024, H=16, D=64) on 8 trn2 NeuronCores.

Sharding: batch x head-group. Core c handles batch b = c//4 and head group
g = c%4 (4 heads = 256 channels). All-bf16 data path (fp32 PSUM accumulate).

Per core:
  - qkv projection for its 4 heads (Q^T/K^T in [d, t] layout, V in [t, d]);
    x^T is DMA'd in T-slices so matmuls start ~3us in and stay dense (keeps
    the PE HAM clock-gate warm at 2.4 GHz)
  - causal flash attention (scores K-major as S^T, exp batched across both
    heads of a pair in one ACT call from a 2-bank PSUM tile, diagonal
    blocks trimmed, row-sums via a ones-column appended to V)
  - AllGather of Y^T across the 4 cores of the same batch, split per
    (pair, T-half) = 4 collectives in bf16, pipelined into attention
  - output projection in transposed layout (out^T[oc, t], N=512 moving),
    2-phase accumulation: pair-0 contribution overlaps pair-1 attention

Host gather: per-core out^T [256, T] -> transpose into [B, T, C] slices.
"""

import os
import numpy as np
import ml_dtypes

import concourse.bass as bass
import concourse.bacc as bacc
import concourse.mybir as mybir
import concourse.tile as tile
from concourse import bass_utils
from concourse.bass import ds, ts
from concourse.bass_interp import get_hw_module

P = 128
B, T, C = 2, 2048, 1024
NH, D = 16, 64
NC = 8          # cores
NG = 4          # head groups (cores per batch)
HL = NH // NG   # heads per core = 4
DL = HL * D     # local channels = 256
NQ = 512        # query tile
F32 = mybir.dt.float32
BF16 = mybir.dt.bfloat16
NPBF16 = ml_dtypes.bfloat16


def _build_body(ctx, tc, io):
    nc = tc.nc
    xt, wq, wk, wv, bq, bk, bv, wp, bp, tri, out, ytl, ytf = io
    mm = nc.tensor.matmul

    pers = ctx.enter_context(tc.tile_pool(name="pers", bufs=1))

    tri_sb = pers.tile([P, P], BF16)
    nc.sync.dma_start(tri_sb[:], tri)

    qt_sb = pers.tile([P, 2, T], BF16)   # pair j; head 2j+1 on partitions 64..127
    kt_sb = pers.tile([P, 2, T], BF16)
    v_sb = pers.tile([P, T // P, HL, D + 1], BF16)  # [l_part, l_chunk, head, d|1]
    ones_stage = pers.tile([P, (T // P) * HL], BF16)
    nc.vector.memset(ones_stage[:], 1.0)
    nc.vector.tensor_copy(
        v_sb[:, :, :, D : D + 1],
        ones_stage[:].rearrange("p (a b) -> p a b", a=T // P)[:, :, :, None],
    )
    # yth[pair]: rows 0..63 head 2p, rows 64..127 head 2p+1 (AG payload layout)
    yth = [pers.tile([P, T], BF16, tag=f"yth{p}", name=f"yth{p}") for p in range(2)]

    xt_sb = pers.tile([P, C // P, T], BF16)
    wq_sb = pers.tile([P, C // P, DL], BF16)
    wk_sb = pers.tile([P, C // P, DL], BF16)
    wv_sb = pers.tile([P, C // P, DL], BF16)
    wp_sb = pers.tile([P, 2 * NG, DL], BF16)  # seg s=4p+r, rows permuted host-side
    acc = pers.tile([P, 2, T], BF16)          # proj phase-A accumulator (out^T)

    bqp = pers.tile([P, 2], F32)
    nc.sync.dma_start(bqp[:], bq.rearrange("(j p) -> p j", p=P))
    bkp = pers.tile([P, 2], F32)
    nc.sync.dma_start(bkp[:], bk.rearrange("(j p) -> p j", p=P))
    bv_row = pers.tile([1, DL], F32)
    nc.sync.dma_start(bv_row[:], bv[None, :])
    bv_bc = pers.tile([P, DL], F32)
    nc.gpsimd.partition_broadcast(bv_bc[:], bv_row[:])
    bpp = pers.tile([P, 2], F32)
    nc.sync.dma_start(bpp[:], bp.rearrange("(o p) -> p o", p=P))

    # weights first, then x^T in T-major slices so tt=0 matmuls start early
    for cc in range(C // P):
        src = slice(P * cc, P * (cc + 1))
        nc.sync.dma_start(wq_sb[:, cc, :], wq[src, :].rearrange("p n -> p n"))
        nc.sync.dma_start(wk_sb[:, cc, :], wk[src, :].rearrange("p n -> p n"))
    for tt in range(T // NQ):
        for cc in range(C // P):
            nc.sync.dma_start(
                xt_sb[:, cc, ts(tt, NQ)],
                xt[P * cc : P * (cc + 1), ts(tt, NQ)].rearrange("p t -> p t"),
            )
    for cc in range(C // P):
        src = slice(P * cc, P * (cc + 1))
        nc.sync.dma_start(wv_sb[:, cc, :], wv[src, :].rearrange("p n -> p n"))
    nc.sync.dma_start(wp_sb[:], wp.rearrange("(s p) n -> p s n", p=P))

    # single PSUM pool: s-groups (4 banks), o (2 banks), gemm chains (2 banks)
    psum = ctx.enter_context(tc.tile_pool(name="psum", bufs=1, space="PSUM"))
    pp = ctx.enter_context(tc.tile_pool(name="pp", bufs=3))
    nrm = ctx.enter_context(tc.tile_pool(name="nrm", bufs=4))
    po = ctx.enter_context(tc.tile_pool(name="po", bufs=4))
    yf = ctx.enter_context(tc.tile_pool(name="yf", bufs=2))

    def qk_tile(w_sb, b_sb, dst, j, tt):
        ps = psum.tile([P, NQ], F32, tag="gemm", name="qk_ps", bufs=2)
        for cc in range(C // P):
            mm(
                ps[:],
                w_sb[:, cc, ts(j, P)],
                xt_sb[:, cc, ts(tt, NQ)],
                start=(cc == 0),
                stop=(cc == C // P - 1),
            )
        nc.vector.tensor_scalar_add(dst[:, j, ts(tt, NQ)], ps[:], b_sb[:, j : j + 1])

    def v_tile(tt):
        ps = psum.tile([P, DL], F32, tag="gemm", name="v_ps", bufs=2)
        for cc in range(C // P):
            mm(
                ps[:],
                xt_sb[:, cc, ts(tt, P)],
                wv_sb[:, cc, :],
                start=(cc == 0),
                stop=(cc == C // P - 1),
            )
        nc.vector.tensor_add(
            v_sb[:, tt, :, 0:D],
            ps[:].rearrange("p (h d) -> p h d", h=HL),
            bv_bc[:].rearrange("p (h d) -> p h d", h=HL),
        )

    def attn_qtile(pair, qt):
        q0 = NQ * qt
        nl = q0 // P + NQ // P  # l-chunks for causal coverage
        o_ps = [
            psum.tile([D + 1, NQ], F32, tag=f"o{hi}", name=f"o_ps{hi}", bufs=1)
            for hi in range(2)
        ]

        def s_stage(lc):
            w0 = max(P * lc - q0, 0)
            s2 = psum.tile([P, 2, NQ], F32, tag="s", name="s2", bufs=2)
            for hi in range(2):
                mm(
                    s2[:, hi, w0:NQ],
                    kt_sb[64 * hi : 64 * hi + 64, pair, ts(lc, P)],
                    qt_sb[64 * hi : 64 * hi + 64, pair, ds(q0 + w0, NQ - w0)],
                    start=True,
                    stop=True,
                    tile_position=(64 * hi, 0),
                )
            return s2

        def pv_stage(lc, s2):
            off = P * lc - q0
            w0 = max(off, 0)
            pt = pp.tile([P, 2, NQ], BF16, tag="p", name="pt")
            nc.scalar.activation(
                pt[:, :, w0:NQ],
                s2[:, :, w0:NQ],
                mybir.ActivationFunctionType.Exp,
                bias=0.0,
                scale=1.0 / np.sqrt(D),
            )
            if off >= 0:
                for hi in range(2):
                    nc.vector.tensor_mul(
                        pt[:, hi, off : off + P],
                        pt[:, hi, off : off + P],
                        tri_sb[:],
                    )
            for hi in range(2):
                mm(
                    o_ps[hi][:, w0:NQ],
                    v_sb[:, lc, 2 * pair + hi, :],
                    pt[:, hi, w0:NQ],
                    start=(lc == 0),
                    stop=(lc == nl - 1),
                )

        # software pipeline: keep one S stage ahead of exp/PV
        prev = s_stage(0)
        for lc in range(1, nl):
            cur = s_stage(lc)
            pv_stage(lc - 1, prev)
            prev = cur
        pv_stage(nl - 1, prev)

        for hi in range(2):
            sums_sb = nrm.tile([1, NQ], F32, tag="sums")
            nc.vector.tensor_copy(sums_sb[:], o_ps[hi][D : D + 1, :])
            rcp = nrm.tile([1, NQ], F32, tag="rcp")
            nc.vector.reciprocal_approx_fast(rcp[:], sums_sb[:])
            bc = nrm.tile([D, NQ], F32, tag="bc")
            nc.gpsimd.partition_broadcast(bc[:], rcp[:])
            nc.vector.tensor_mul(
                yth[pair][64 * hi : 64 * hi + 64, ds(q0, NQ)],
                o_ps[hi][0:D, :],
                bc[:],
            )

    def ship(pair, half):
        # DMA this (pair, T-half) of Y^T to HBM, AllGather within the batch
        nc.sync.dma_start(ytl[pair][half][:], yth[pair][:, ts(half, T // 2)])
        nc.gpsimd.collective_compute(
            "AllGather",
            mybir.AluOpType.bypass,
            replica_groups=[[0, 1, 2, 3], [4, 5, 6, 7]],
            ins=[ytl[pair][half][:]],
            outs=[ytf[pair][half][:]],
        )

    def proj_a(half):
        # pair-0 contribution: 4 rank-segs -> acc (bf16) + bias
        y0 = yf.tile([P, NG, T // 2], BF16, tag="y0", name="y0")
        nc.sync.dma_start(
            y0[:], ytf[0][half].rearrange("(g p) t -> p g t", p=P)
        )
        for oc in range(2):
            for s in range(2):  # 512-col subtiles of the half
                t0 = half * (T // 2) + s * NQ
                ps = psum.tile([P, NQ], F32, tag="gemm", name="prA_ps", bufs=2)
                for g in range(NG):
                    mm(
                        ps[:],
                        wp_sb[:, g, ts(oc, P)],
                        y0[:, g, ts(s, NQ)],
                        start=(g == 0),
                        stop=(g == NG - 1),
                    )
                nc.vector.tensor_scalar_add(
                    acc[:, oc, ds(t0, NQ)], ps[:], bpp[:, oc : oc + 1]
                )

    def proj_b(half):
        y1 = yf.tile([P, NG, T // 2], BF16, tag="y1", name="y1")
        nc.sync.dma_start(
            y1[:], ytf[1][half].rearrange("(g p) t -> p g t", p=P)
        )
        for oc in range(2):
            for s in range(2):
                t0 = half * (T // 2) + s * NQ
                ps = psum.tile([P, NQ], F32, tag="gemm", name="prB_ps", bufs=2)
                for g in range(NG):
                    mm(
                        ps[:],
                        wp_sb[:, NG + g, ts(oc, P)],
                        y1[:, g, ts(s, NQ)],
                        start=(g == 0),
                        stop=(g == NG - 1),
                    )
                ot = po.tile([P, NQ], F32, tag="ot")
                nc.vector.tensor_add(ot[:], ps[:], acc[:, oc, ds(t0, NQ)])
                nc.sync.dma_start(out[ts(oc, P), ds(t0, NQ)], ot[:])

    # ---------------- program ----------------
    for tt in range(T // NQ):
        qk_tile(wk_sb, bkp, kt_sb, 0, tt)
        qk_tile(wq_sb, bqp, qt_sb, 0, tt)

    for qt in range(T // NQ):
        for tt in range(4 * qt, 4 * qt + 4):
            v_tile(tt)
        attn_qtile(0, qt)
        if qt == 1:
            ship(0, 0)
    ship(0, 1)

    for tt in range(T // NQ):
        qk_tile(wk_sb, bkp, kt_sb, 1, tt)
        qk_tile(wq_sb, bqp, qt_sb, 1, tt)

    for qt in range(T // NQ):
        attn_qtile(1, qt)
        if qt == 1:
            ship(1, 0)
    ship(1, 1)

    for half in range(2):
        proj_a(half)
    for half in range(2):
        proj_b(half)


def build_program():
    nc = bacc.Bacc(
        "TRN2",
        target_bir_lowering=False,
        debug=False,
        enable_asserts=False,
        num_devices=NC,
    )
    xt = nc.dram_tensor("xt", [C, T], BF16, kind="ExternalInput").ap()
    wq = nc.dram_tensor("wq", [C, DL], BF16, kind="ExternalInput").ap()
    wk = nc.dram_tensor("wk", [C, DL], BF16, kind="ExternalInput").ap()
    wv = nc.dram_tensor("wv", [C, DL], BF16, kind="ExternalInput").ap()
    bq = nc.dram_tensor("bq", [DL], F32, kind="ExternalInput").ap()
    bk = nc.dram_tensor("bk", [DL], F32, kind="ExternalInput").ap()
    bv = nc.dram_tensor("bv", [DL], F32, kind="ExternalInput").ap()
    wp = nc.dram_tensor("wp", [C, DL], BF16, kind="ExternalInput").ap()
    bp = nc.dram_tensor("bp", [DL], F32, kind="ExternalInput").ap()
    tri = nc.dram_tensor("tri", [P, P], BF16, kind="ExternalInput").ap()
    out = nc.dram_tensor("out", [DL, T], F32, kind="ExternalOutput").ap()
    ytl = [
        [
            nc.dram_tensor(f"ytl{p}_{h}", [P, T // 2], BF16, kind="Internal").ap()
            for h in range(2)
        ]
        for p in range(2)
    ]
    ytf = [
        [
            nc.dram_tensor(
                f"ytf{p}_{h}", [NG * P, T // 2], BF16, kind="Internal"
            ).ap()
            for h in range(2)
        ]
        for p in range(2)
    ]
    io = (xt, wq, wk, wv, bq, bk, bv, wp, bp, tri, out, ytl, ytf)
    with tile.TileContext(nc) as tc:
        import contextlib

        with contextlib.ExitStack() as ctx:
            _build_body(ctx, tc, io)
    nc.compile()
    return nc


def _stage_wp(W_proj, g):
    """wp rows permuted to match ytf row order (rank-major, per pair).

    ytf[p][half] rows = rank r in the batch group at [128r : 128r+128],
    carrying heads (4r + 2p) on rows 0..63 and (4r + 2p + 1) on 64..127.
    Seg s = 4p + r of the staged wp holds W_proj rows for those two heads.
    """
    cols = slice(DL * g, DL * (g + 1))
    Wl = W_proj[:, cols]  # [C, DL]
    segs = []
    for p in range(2):
        for r in range(NG):
            h0 = 4 * r + 2 * p
            segs.append(Wl[64 * h0 : 64 * h0 + 128, :])
    return np.ascontiguousarray(np.concatenate(segs, axis=0)).astype(NPBF16)


def make_in_maps(x, W_attn, b_attn, W_proj, b_proj):
    # scores are computed transposed (S^T[l, q]); position (l', q'') in a
    # diagonal 128x128 block is causally valid iff q'' >= l' -> upper-tri mask
    tri_np = np.triu(np.ones((P, P), dtype=np.float32)).astype(NPBF16)
    x = np.asarray(x, dtype=np.float32)
    W_attn = np.asarray(W_attn, dtype=np.float32)
    b_attn = np.asarray(b_attn, dtype=np.float32)
    W_proj = np.asarray(W_proj, dtype=np.float32)
    b_proj = np.asarray(b_proj, dtype=np.float32)
    in_maps = []
    for c in range(NC):
        b, g = divmod(c, NG)
        cols = slice(DL * g, DL * (g + 1))
        in_maps.append(
            {
                "xt": np.ascontiguousarray(x[b].T).astype(NPBF16),
                "wq": np.ascontiguousarray(W_attn[:, cols]).astype(NPBF16),
                "wk": np.ascontiguousarray(W_attn[:, C:][:, cols]).astype(NPBF16),
                "wv": np.ascontiguousarray(W_attn[:, 2 * C :][:, cols]).astype(
                    NPBF16
                ),
                "bq": np.ascontiguousarray(b_attn[cols]),
                "bk": np.ascontiguousarray(b_attn[C:][cols]),
                "bv": np.ascontiguousarray(b_attn[2 * C :][cols]),
                "wp": _stage_wp(W_proj, g),
                "bp": np.ascontiguousarray(b_proj[cols]),
                "tri": tri_np,
            }
        )
    return in_maps


_NC_CACHE = {}


def _install_ntff_hook():
    """Recreate the missing antenv.axon_hooks module so
    run_bass_kernel_spmd(trace=True) can capture NTFF profiles under axon."""
    import sys
    import types

    if "antenv.axon_hooks" in sys.modules:
        return True
    try:
        from trn_agent_boot.trn_boot import _ntff_profile_via_ctypes

        hook = _ntff_profile_via_ctypes("/opt/axon/libaxon_pjrt.so")
        if hook is None:
            return False
        mod = types.ModuleType("antenv.axon_hooks")
        mod.get_axon_ntff_profile_hook = lambda: hook
        mod.set_axon_ntff_profile_hook = lambda h: None
        sys.modules["antenv.axon_hooks"] = mod
        import antenv

        antenv.axon_hooks = mod
        # keep trace artifacts local (no fish bucket in this container)
        bass_utils.upload_artifacts = lambda tmpdir: tmpdir
        return True
    except Exception:
        return False


def _get_program():
    if "nc" not in _NC_CACHE:
        nc = build_program()
        nc.m = get_hw_module(nc.m)
        _NC_CACHE["nc"] = nc
    return _NC_CACHE["nc"]


def kernel(x, W_attn, b_attn, W_proj, b_proj):
    nc = _get_program()
    in_maps = make_in_maps(x, W_attn, b_attn, W_proj, b_proj)
    trace = bool(int(os.environ.get("KERNEL_TRACE", "0")))
    if trace:
        trace = _install_ntff_hook()
    res = bass_utils.run_bass_kernel_spmd(
        nc,
        in_maps,
        core_ids=list(range(NC)),
        trace=trace,
        trace_cores=list(range(NC)) if trace else None,
    )
    if trace:
        _NC_CACHE["last_results"] = res
        if res.exec_time_ns is not None:
            print(f"HW exec time: {res.exec_time_ns} ns")
            if res.instructions_and_trace is not None:
                print(f"trace: {res.instructions_and_trace[1]}")
    out = np.empty((B, T, C), dtype=np.float32)
    for c in range(NC):
        b, g = divmod(c, NG)
        out[b, :, DL * g : DL * (g + 1)] = res.results[c]["out"].T
    return out


# revision 5
# speedup vs baseline: 1.5927x; 1.0516x over previous
"""Causal self-attention (B=2, T=2048, C=1024, H=16, D=64) on 8 trn2 NeuronCores.

Sharding: batch x head-group. Core c handles batch b = c//4 and head group
g = c%4 (4 heads = 256 channels). All-bf16 data path (fp32 PSUM accumulate).

Per core:
  - warmup matmul burst at t=0 (overlapping input DMA) so the PE HAM clock
    gate flips to 2.4 GHz before real work, and stays there
  - qkv projection for its 4 heads (Q^T/K^T in [d, t] layout, V in [t, d]);
    x^T DMA'd in four 1MB T-slices, weights in single DMAs
  - causal flash attention (scores K-major as S^T, exp batched across both
    heads of a pair in one ACT call from a 2-bank PSUM tile, diagonal
    blocks trimmed, row-sums via a ones-column appended to V)
  - AllGather of Y^T across all 8 cores (the 8-rank on-chip path is much
    faster than a 4-rank ring), split per (pair, T-half) = 4 collectives
    in bf16, pipelined into attention; proj weight rows for the foreign
    batch are staged as zeros so one SPMD program works for both batches
  - output projection in transposed layout (out^T[oc, t], N=512 moving),
    2-phase accumulation interleaved into pair-1 attention

Host gather: per-core out^T [256, T] -> transpose into [B, T, C] slices.
"""

import os
import numpy as np
import ml_dtypes

import concourse.bass as bass
import concourse.bacc as bacc
import concourse.mybir as mybir
import concourse.tile as tile
from concourse import bass_utils
from concourse.bass import ds, ts
from concourse.bass_interp import get_hw_module

P = 128
B, T, C = 2, 2048, 1024
NH, D = 16, 64
NC = 8          # cores
NG = 4          # head groups (cores per batch)
HL = NH // NG   # heads per core = 4
DL = HL * D     # local channels = 256
NQ = 512        # query tile
F32 = mybir.dt.float32
BF16 = mybir.dt.bfloat16
NPBF16 = ml_dtypes.bfloat16

AG_WORLD = 8    # 8: one 8-rank AllGather (foreign batch rows zero-weighted)
NSEG = 2 * NG if AG_WORLD == 8 else NG  # proj row-segs per pair
N_WARMUP = 28   # dummy matmuls to warm the HAM clock gate


def _build_body(ctx, tc, io):
    nc = tc.nc
    xt, wq, wk, wv, bq, bk, bv, wp, bp, tri, out, ytl, ytf = io
    mm = nc.tensor.matmul

    pers = ctx.enter_context(tc.tile_pool(name="pers", bufs=1))
    psum = ctx.enter_context(tc.tile_pool(name="psum", bufs=1, space="PSUM"))
    pp = ctx.enter_context(tc.tile_pool(name="pp", bufs=3))
    nrm = ctx.enter_context(tc.tile_pool(name="nrm", bufs=4))
    po = ctx.enter_context(tc.tile_pool(name="po", bufs=4))
    yf = ctx.enter_context(tc.tile_pool(name="yf", bufs=2))

    # ---- HAM warmup: dense dummy matmuls while input DMAs stream in ----
    dummy = pers.tile([P, NQ], BF16)
    nc.vector.memset(dummy[:], 0.0)
    for _ in range(N_WARMUP):
        wps = psum.tile([P, NQ], F32, tag="gemm", name="warm_ps", bufs=2)
        mm(wps[:], dummy[:, 0:P], dummy[:], start=True, stop=True)

    tri_sb = pers.tile([P, P], BF16)
    qt_sb = pers.tile([P, 2, T], BF16)   # pair j; head 2j+1 on partitions 64..127
    kt_sb = pers.tile([P, 2, T], BF16)
    v_sb = pers.tile([P, T // P, HL, D + 1], BF16)  # [l_part, l_chunk, head, d|1]
    ones_stage = pers.tile([P, (T // P) * HL], BF16)
    nc.vector.memset(ones_stage[:], 1.0)
    nc.vector.tensor_copy(
        v_sb[:, :, :, D : D + 1],
        ones_stage[:].rearrange("p (a b) -> p a b", a=T // P)[:, :, :, None],
    )
    # yth[pair]: rows 0..63 head 2p, rows 64..127 head 2p+1 (AG payload layout)
    yth = [pers.tile([P, T], BF16, tag=f"yth{p}", name=f"yth{p}") for p in range(2)]

    xt_sb = pers.tile([P, C // P, T], BF16)
    wq_sb = pers.tile([P, C // P, DL], BF16)
    wk_sb = pers.tile([P, C // P, DL], BF16)
    wv_sb = pers.tile([P, C // P, DL], BF16)
    wp_sb = pers.tile([P, 2 * NSEG, DL], BF16)  # seg s = NSEG*p + r
    acc = pers.tile([P, 2, T], BF16)            # proj phase-A accumulator (out^T)

    bqp = pers.tile([P, 2], F32)
    bkp = pers.tile([P, 2], F32)
    bv_row = pers.tile([1, DL], F32)
    bv_bc = pers.tile([P, DL], F32)
    bpp = pers.tile([P, 2], F32)

    # ---- input DMAs: big transfers, ordered so tt=0 matmuls start early ----
    nc.sync.dma_start(wk_sb[:], wk.rearrange("(c p) n -> p c n", p=P))
    nc.sync.dma_start(wq_sb[:], wq.rearrange("(c p) n -> p c n", p=P))
    for tt in range(T // NQ):
        nc.sync.dma_start(
            xt_sb[:, :, ts(tt, NQ)],
            xt[:, ts(tt, NQ)].rearrange("(c p) t -> p c t", p=P),
        )
        if tt == 0:
            nc.sync.dma_start(bqp[:], bq.rearrange("(j p) -> p j", p=P))
            nc.sync.dma_start(bkp[:], bk.rearrange("(j p) -> p j", p=P))
            nc.sync.dma_start(bv_row[:], bv[None, :])
            nc.gpsimd.partition_broadcast(bv_bc[:], bv_row[:])
            nc.sync.dma_start(tri_sb[:], tri)
    nc.sync.dma_start(wv_sb[:], wv.rearrange("(c p) n -> p c n", p=P))
    nc.sync.dma_start(wp_sb[:], wp.rearrange("(s p) n -> p s n", p=P))
    nc.sync.dma_start(bpp[:], bp.rearrange("(o p) -> p o", p=P))

    def qk_tile(w_sb, b_sb, dst, j, tt):
        ps = psum.tile([P, NQ], F32, tag="gemm", name="qk_ps", bufs=2)
        for cc in range(C // P):
            mm(
                ps[:],
                w_sb[:, cc, ts(j, P)],
                xt_sb[:, cc, ts(tt, NQ)],
                start=(cc == 0),
                stop=(cc == C // P - 1),
            )
        nc.vector.tensor_scalar_add(dst[:, j, ts(tt, NQ)], ps[:], b_sb[:, j : j + 1])

    def v_tile(tt):
        ps = psum.tile([P, DL], F32, tag="gemm", name="v_ps", bufs=2)
        for cc in range(C // P):
            mm(
                ps[:],
                xt_sb[:, cc, ts(tt, P)],
                wv_sb[:, cc, :],
                start=(cc == 0),
                stop=(cc == C // P - 1),
            )
        nc.vector.tensor_add(
            v_sb[:, tt, :, 0:D],
            ps[:].rearrange("p (h d) -> p h d", h=HL),
            bv_bc[:].rearrange("p (h d) -> p h d", h=HL),
        )

    def attn_qtile(pair, qt):
        q0 = NQ * qt
        nl = q0 // P + NQ // P  # l-chunks for causal coverage
        o_ps = [
            psum.tile([D + 1, NQ], F32, tag=f"o{hi}", name=f"o_ps{hi}", bufs=1)
            for hi in range(2)
        ]

        def s_stage(lc):
            w0 = max(P * lc - q0, 0)
            s2 = psum.tile([P, 2, NQ], F32, tag="s", name="s2", bufs=2)
            for hi in range(2):
                mm(
                    s2[:, hi, w0:NQ],
                    kt_sb[64 * hi : 64 * hi + 64, pair, ts(lc, P)],
                    qt_sb[64 * hi : 64 * hi + 64, pair, ds(q0 + w0, NQ - w0)],
                    start=True,
                    stop=True,
                    tile_position=(64 * hi, 0),
                )
            return s2

        def pv_stage(lc, s2):
            off = P * lc - q0
            w0 = max(off, 0)
            pt = pp.tile([P, 2, NQ], BF16, tag="p", name="pt")
            nc.scalar.activation(
                pt[:, :, w0:NQ],
                s2[:, :, w0:NQ],
                mybir.ActivationFunctionType.Exp,
                bias=0.0,
                scale=1.0 / np.sqrt(D),
            )
            if off >= 0:
                for hi in range(2):
                    nc.vector.tensor_mul(
                        pt[:, hi, off : off + P],
                        pt[:, hi, off : off + P],
                        tri_sb[:],
                    )
            for hi in range(2):
                mm(
                    o_ps[hi][:, w0:NQ],
                    v_sb[:, lc, 2 * pair + hi, :],
                    pt[:, hi, w0:NQ],
                    start=(lc == 0),
                    stop=(lc == nl - 1),
                )

        # software pipeline: keep one S stage ahead of exp/PV
        prev = s_stage(0)
        for lc in range(1, nl):
            cur = s_stage(lc)
            pv_stage(lc - 1, prev)
            prev = cur
        pv_stage(nl - 1, prev)

        for hi in range(2):
            sums_sb = nrm.tile([1, NQ], F32, tag="sums")
            nc.vector.tensor_copy(sums_sb[:], o_ps[hi][D : D + 1, :])
            rcp = nrm.tile([1, NQ], F32, tag="rcp")
            nc.vector.reciprocal_approx_fast(rcp[:], sums_sb[:])
            bc = nrm.tile([D, NQ], F32, tag="bc")
            nc.gpsimd.partition_broadcast(bc[:], rcp[:])
            nc.vector.tensor_mul(
                yth[pair][64 * hi : 64 * hi + 64, ds(q0, NQ)],
                o_ps[hi][0:D, :],
                bc[:],
            )

    if AG_WORLD == 8:
        replica_groups = [list(range(NC))]
    else:
        replica_groups = [[0, 1, 2, 3], [4, 5, 6, 7]]

    def ship(pair, half):
        # DMA this (pair, T-half) of Y^T to HBM and AllGather it
        nc.sync.dma_start(ytl[pair][half][:], yth[pair][:, ts(half, T // 2)])
        nc.gpsimd.collective_compute(
            "AllGather",
            mybir.AluOpType.bypass,
            replica_groups=replica_groups,
            ins=[ytl[pair][half][:]],
            outs=[ytf[pair][half][:]],
        )

    def proj_half(pair, half):
        # one pair's contribution to out^T for one T-half
        y = yf.tile([P, NSEG, T // 2], BF16, tag=f"y{pair}", name=f"y{pair}")
        nc.sync.dma_start(
            y[:], ytf[pair][half].rearrange("(g p) t -> p g t", p=P)
        )
        for oc in range(2):
            for s in range(2):  # 512-col subtiles of the half
                t0 = half * (T // 2) + s * NQ
                ps = psum.tile([P, NQ], F32, tag="gemm", name="pr_ps", bufs=2)
                for g in range(NSEG):
                    mm(
                        ps[:],
                        wp_sb[:, NSEG * pair + g, ts(oc, P)],
                        y[:, g, ts(s, NQ)],
                        start=(g == 0),
                        stop=(g == NSEG - 1),
                    )
                if pair == 0:
                    nc.vector.tensor_scalar_add(
                        acc[:, oc, ds(t0, NQ)], ps[:], bpp[:, oc : oc + 1]
                    )
                else:
                    ot = po.tile([P, NQ], F32, tag="ot")
                    nc.vector.tensor_add(ot[:], ps[:], acc[:, oc, ds(t0, NQ)])
                    nc.sync.dma_start(out[ts(oc, P), ds(t0, NQ)], ot[:])

    # ---------------- program ----------------
    for tt in range(T // NQ):
        qk_tile(wk_sb, bkp, kt_sb, 0, tt)
        qk_tile(wq_sb, bqp, qt_sb, 0, tt)

    for qt in range(T // NQ):
        for tt in range(4 * qt, 4 * qt + 4):
            v_tile(tt)
        attn_qtile(0, qt)
        if qt == 1:
            ship(0, 0)
    ship(0, 1)

    for tt in range(T // NQ):
        qk_tile(wk_sb, bkp, kt_sb, 1, tt)
        qk_tile(wq_sb, bqp, qt_sb, 1, tt)

    attn_qtile(1, 0)
    attn_qtile(1, 1)
    ship(1, 0)
    attn_qtile(1, 2)
    proj_half(0, 0)
    attn_qtile(1, 3)
    proj_half(0, 1)
    ship(1, 1)
    proj_half(1, 0)
    proj_half(1, 1)


def build_program():
    nc = bacc.Bacc(
        "TRN2",
        target_bir_lowering=False,
        debug=False,
        enable_asserts=False,
        num_devices=NC,
    )
    xt = nc.dram_tensor("xt", [C, T], BF16, kind="ExternalInput").ap()
    wq = nc.dram_tensor("wq", [C, DL], BF16, kind="ExternalInput").ap()
    wk = nc.dram_tensor("wk", [C, DL], BF16, kind="ExternalInput").ap()
    wv = nc.dram_tensor("wv", [C, DL], BF16, kind="ExternalInput").ap()
    bq = nc.dram_tensor("bq", [DL], F32, kind="ExternalInput").ap()
    bk = nc.dram_tensor("bk", [DL], F32, kind="ExternalInput").ap()
    bv = nc.dram_tensor("bv", [DL], F32, kind="ExternalInput").ap()
    wp = nc.dram_tensor("wp", [2 * NSEG * P, DL], BF16, kind="ExternalInput").ap()
    bp = nc.dram_tensor("bp", [DL], F32, kind="ExternalInput").ap()
    tri = nc.dram_tensor("tri", [P, P], BF16, kind="ExternalInput").ap()
    out = nc.dram_tensor("out", [DL, T], F32, kind="ExternalOutput").ap()
    ytl = [
        [
            nc.dram_tensor(f"ytl{p}_{h}", [P, T // 2], BF16, kind="Internal").ap()
            for h in range(2)
        ]
        for p in range(2)
    ]
    ytf = [
        [
            nc.dram_tensor(
                f"ytf{p}_{h}",
                [AG_WORLD * P, T // 2],
                BF16,
                kind="Internal",
                addr_space="Shared",
            ).ap()
            for h in range(2)
        ]
        for p in range(2)
    ]
    io = (xt, wq, wk, wv, bq, bk, bv, wp, bp, tri, out, ytl, ytf)
    with tile.TileContext(nc) as tc:
        import contextlib

        with contextlib.ExitStack() as ctx:
            _build_body(ctx, tc, io)
    nc.compile()
    return nc


def _stage_wp(W_proj, b, g):
    """wp rows permuted to match ytf row order (rank-major, per pair).

    With AG_WORLD == 8, ytf[p][half] row block r (128 rows) comes from core
    r, carrying batch r//4, group r%4, heads (4*(r%4) + 2p + {0,1}). Blocks
    of the foreign batch get zero weights so one SPMD program serves both
    batches. With AG_WORLD == 4, blocks are the 4 same-batch ranks.
    """
    cols = slice(DL * g, DL * (g + 1))
    Wl = W_proj[:, cols]  # [C, DL]
    segs = []
    for p in range(2):
        if AG_WORLD == 8:
            for r in range(NC):
                if r // NG == b:
                    h0 = 4 * (r % NG) + 2 * p
                    segs.append(Wl[64 * h0 : 64 * h0 + 128, :])
                else:
                    segs.append(np.zeros((P, DL), np.float32))
        else:
            for r in range(NG):
                h0 = 4 * r + 2 * p
                segs.append(Wl[64 * h0 : 64 * h0 + 128, :])
    return np.ascontiguousarray(np.concatenate(segs, axis=0)).astype(NPBF16)


def make_in_maps(x, W_attn, b_attn, W_proj, b_proj):
    # scores are computed transposed (S^T[l, q]); position (l', q'') in a
    # diagonal 128x128 block is causally valid iff q'' >= l' -> upper-tri mask
    tri_np = np.triu(np.ones((P, P), dtype=np.float32)).astype(NPBF16)
    x = np.asarray(x, dtype=np.float32)
    W_attn = np.asarray(W_attn, dtype=np.float32)
    b_attn = np.asarray(b_attn, dtype=np.float32)
    W_proj = np.asarray(W_proj, dtype=np.float32)
    b_proj = np.asarray(b_proj, dtype=np.float32)
    in_maps = []
    for c in range(NC):
        b, g = divmod(c, NG)
        cols = slice(DL * g, DL * (g + 1))
        in_maps.append(
            {
                "xt": np.ascontiguousarray(x[b].T).astype(NPBF16),
                "wq": np.ascontiguousarray(W_attn[:, cols]).astype(NPBF16),
                "wk": np.ascontiguousarray(W_attn[:, C:][:, cols]).astype(NPBF16),
                "wv": np.ascontiguousarray(W_attn[:, 2 * C :][:, cols]).astype(
                    NPBF16
                ),
                "bq": np.ascontiguousarray(b_attn[cols]),
                "bk": np.ascontiguousarray(b_attn[C:][cols]),
                "bv": np.ascontiguousarray(b_attn[2 * C :][cols]),
                "wp": _stage_wp(W_proj, b, g),
                "bp": np.ascontiguousarray(b_proj[cols]),
                "tri": tri_np,
            }
        )
    return in_maps


_NC_CACHE = {}


def _install_ntff_hook():
    """Recreate the missing antenv.axon_hooks module so
    run_bass_kernel_spmd(trace=True) can capture NTFF profiles under axon."""
    import sys
    import types

    if "antenv.axon_hooks" in sys.modules:
        return True
    try:
        from trn_agent_boot.trn_boot import _ntff_profile_via_ctypes

        hook = _ntff_profile_via_ctypes("/opt/axon/libaxon_pjrt.so")
        if hook is None:
            return False
        mod = types.ModuleType("antenv.axon_hooks")
        mod.get_axon_ntff_profile_hook = lambda: hook
        mod.set_axon_ntff_profile_hook = lambda h: None
        sys.modules["antenv.axon_hooks"] = mod
        import antenv

        antenv.axon_hooks = mod
        # keep trace artifacts local (no fish bucket in this container)
        bass_utils.upload_artifacts = lambda tmpdir: tmpdir
        return True
    except Exception:
        return False


def _get_program():
    if "nc" not in _NC_CACHE:
        nc = build_program()
        nc.m = get_hw_module(nc.m)
        _NC_CACHE["nc"] = nc
    return _NC_CACHE["nc"]


def kernel(x, W_attn, b_attn, W_proj, b_proj):
    nc = _get_program()
    in_maps = make_in_maps(x, W_attn, b_attn, W_proj, b_proj)
    trace = bool(int(os.environ.get("KERNEL_TRACE", "0")))
    if trace:
        trace = _install_ntff_hook()
    res = bass_utils.run_bass_kernel_spmd(
        nc,
        in_maps,
        core_ids=list(range(NC)),
        trace=trace,
        trace_cores=list(range(NC)) if trace else None,
    )
    if trace:
        _NC_CACHE["last_results"] = res
        if res.exec_time_ns is not None:
            print(f"HW exec time: {res.exec_time_ns} ns")
            if res.instructions_and_trace is not None:
                print(f"trace: {res.instructions_and_trace[1]}")
    out = np.empty((B, T, C), dtype=np.float32)
    for c in range(NC):
        b, g = divmod(c, NG)
        out[b, :, DL * g : DL * (g + 1)] = res.results[c]["out"].T
    return out
